# revision 23
# baseline (speedup 1.0000x reference)
"""Trainium2 Bass kernel for a 2-layer GAT (nn_GAT_50586124812836).

kernel(**inputs) takes the FULL inputs from reference.setup_inputs() and
returns the full [50000, 32] float32 output. Internally: destination-node
sharding across 8 NeuronCores, edges sorted by dst and padded per 128-dst
tile; per layer a dense phase computes h/alpha per shard, an AllGather
publishes a packed bf16 node table, and an edge phase uses SWDGE dma_gather
(int16 indices) plus one-hot PE matmuls to do the segment softmax and
weighted aggregation on-chip. Layer-2's table packs 2 nodes per 256B row
(gather groups split by src parity), halving that AllGather. Leaky-relu /
exp run on the Activation engine; per-edge exp lands directly in the
matmul rhs tile.
"""
import math
from dataclasses import dataclass

import numpy as np
import ml_dtypes

import concourse.mybir as mybir
from concourse import bass
from concourse.bass import AP, MemorySpace
from concourse import ap_utils
from concourse._compat import exact_div

import concourse.bass as bass
import concourse.tile as tile
from concourse import bacc, mybir
from concourse.masks import make_identity
from concourse.library_config import mlp

BF16 = mybir.dt.bfloat16
F32 = mybir.dt.float32
I16 = mybir.dt.int16
P = 128
Alu = mybir.AluOpType
Act = mybir.ActivationFunctionType
NEG_SLOPE = 0.2
BF = ml_dtypes.bfloat16


def dma_gather_raw(
    gp,                       # nc.gpsimd
    out_ap: AP,
    in_ap: AP,
    idxs_ap: AP,
    num_idxs: int,
    elem_size: int,
    elem_step: int,
    queue_num: int = 0,
    single_packet: bool = True,
):
    assert idxs_ap.dtype == mybir.dt.int16
    assert in_ap.space == MemorySpace.DRAM
    assert idxs_ap.space == MemorySpace.SBUF
    assert out_ap.space == MemorySpace.SBUF
    assert in_ap.dtype == out_ap.dtype
    dtsz = mybir.dt.size(in_ap.dtype)
    stride_bytes = elem_step * dtsz
    stride_bytes_256 = exact_div(stride_bytes, 256)
    assert 0 < stride_bytes_256 < 256
    assert ap_utils.ap_is_contiguous(in_ap.ap[1:])
    assert ap_utils.ap_is_contiguous(out_ap.ap[1:])
    assert ap_utils.ap_is_contiguous(idxs_ap.ap[1:])
    assert in_ap.ap[0][0] == elem_step
    assert in_ap.ap[-1][1] == elem_size
    assert out_ap.ap[-1][1] == elem_size
    assert num_idxs % 128 == 0
    assert out_ap.ap[0][1] * out_ap.ap[1][1] == num_idxs

    _in_ap = gp.lower_ap_dma(in_ap, for_custom_bir_dma=True)
    _idxs_ap = gp.lower_ap(idxs_ap)
    _out_ap = gp.lower_ap(out_ap)
    inst = gp.add_instruction(
        mybir.InstDMAGatherAnt(
            name=gp.bass.get_next_instruction_name(),
            ins=[
                *_in_ap,
                _idxs_ap,
                gp.lower_val_access(gp.to_reg(num_idxs)),
            ],
            outs=[_out_ap],
            transpose=False,
            num_idxs=num_idxs,
            elem_size=elem_size,
            stride_bytes_256=stride_bytes_256,
            gen_mode=0,
            single_packet=single_packet,
            queue_num=queue_num,
            sbuf_tokens_per_rank=0,
            sbuf_free_dim_per_rank=0,
            sbuf_free_dim_pad_per_rank=0,
            sbuf_byte_offset=0,
        )
    )
    return inst


@dataclass
class Cfg:
    N: int = 50000
    NC: int = 8
    F: int = 512
    H1: int = 8
    HD: int = 8
    D2: int = 32
    CH: int = 10         # chunks per (tile, class)
    TB: int = 7          # tiles per batch

    def __post_init__(self):
        self.D1 = self.H1 * self.HD
        assert self.N % self.NC == 0
        self.SHARD = self.N // self.NC
        self.TILES = math.ceil(self.SHARD / P)
        self.SHARD_PAD = self.TILES * P
        self.V = self.NC * self.SHARD_PAD
        self.VH = self.V // 2          # layer-1 lo/hi table split
        self.VH2 = self.V // 2         # layer-2 packed table rows
        assert self.VH < 32768
        assert self.TILES % self.TB == 0
        self.NB = self.TILES // self.TB
        assert self.F % P == 0
        self.KC = self.F // P
        self.ROW1 = self.D1 + self.H1           # 72
        self.ROW2 = self.D2 + 1                 # 33
        self.DE1 = self.D1 + 2 * self.H1        # 80: [h | as | ad]
        self.DE2 = self.D2 + 2                  # 34: [h2 | as2 | ad2]
        assert self.SHARD_PAD % 2 == 0
        self.SHP2 = self.SHARD_PAD // 2
        self.KBH = self.TB * self.CH            # chunks per class per batch
        self.NIDX = self.KBH * P                # gather idxs per instr
        self.WCOLS = self.KBH * 8               # wrapped idx cols per batch


def build_program(cfg: Cfg):
    nc = bacc.Bacc("TRN2", target_bir_lowering=False, debug=False,
                   num_devices=cfg.NC, num_swdge_queues=4)
    dt = nc.dram_tensor
    xT = dt("xT", [cfg.F, cfg.SHARD_PAD], BF16, kind="ExternalInput")
    w1 = dt("w1", [P, cfg.KC * cfg.DE1], BF16, kind="ExternalInput")
    w2 = dt("w2", [cfg.D1, cfg.DE2], BF16, kind="ExternalInput")
    b1r = dt("b1r", [P, cfg.D1], F32, kind="ExternalInput")
    b2r = dt("b2r", [P, cfg.D2], F32, kind="ExternalInput")
    srcW = {}
    dstW = {}
    dstl = {}
    for li, (ca, cb) in ((1, ("lo", "hi")), (2, ("p0", "p1"))):
        for s in (ca, cb):
            k = f"{li}{s}"
            srcW[k] = dt(f"srcW_{k}", [P, cfg.NB, cfg.WCOLS], I16,
                         kind="ExternalInput")
            dstW[k] = dt(f"dstW_{k}", [P, cfg.NB, cfg.WCOLS], I16,
                         kind="ExternalInput")
            dstl[k] = dt(f"dstl_{k}", [P, cfg.NB, cfg.KBH], F32,
                         kind="ExternalInput")
    out = dt("out", [cfg.SHARD_PAD, cfg.D2], F32, kind="ExternalOutput")

    gspace = "Shared" if cfg.NC > 4 else "Local"
    ha1_sh = dt("ha1_sh", [cfg.SHARD_PAD, P], BF16, kind="Internal")
    ha1_full = dt("ha1_full", [cfg.V, P], BF16, kind="Internal",
                  addr_space=gspace)
    ha2_sh = dt("ha2_sh", [cfg.SHP2, P], BF16, kind="Internal")
    ha2_full = dt("ha2_full", [cfg.VH2, P], BF16, kind="Internal",
                  addr_space=gspace)
    ad2 = dt("ad2", [cfg.SHARD_PAD, P], BF16, kind="Internal")

    rg = [list(range(cfg.NC))]

    with tile.TileContext(nc) as tc:
        cpool_cm = tc.tile_pool(name="consts", bufs=1)
        cpool = cpool_cm.__enter__()
        nc.gpsimd.load_library(mlp)
        w1s = cpool.tile([P, cfg.KC, cfg.DE1], BF16)
        nc.sync.dma_start(w1s[:], w1[:].rearrange("p (k d) -> p k d", k=cfg.KC))
        w2s = cpool.tile([cfg.D1, cfg.DE2], BF16)
        nc.sync.dma_start(w2s[:], w2[:])
        b1_s = cpool.tile([P, cfg.D1], F32)
        nc.sync.dma_start(b1_s[:], b1r[:])
        b2_s = cpool.tile([P, cfg.D2], F32)
        nc.sync.dma_start(b2_s[:], b2r[:])
        iota_i = cpool.tile([P, P], mybir.dt.int32)
        nc.gpsimd.iota(iota_i[:], pattern=[[1, P]], base=0,
                       channel_multiplier=0)
        iota_bf = cpool.tile([P, P], BF16)
        nc.vector.tensor_copy(iota_bf[:], iota_i[:])
        ident = cpool.tile([P, P], BF16)
        make_identity(nc, ident[:])
        ZT = cpool.tile([cfg.D1, cfg.TILES * P], BF16)

        # ------- Phase 1: [h1 | as1 | ad1] = x @ W1cat, batched per TB -------
        # ha1_sh row layout: [h(64) | as(8) | ad(8) | pad]; the dst-alpha
        # gather reads cols 72:80 locally, the AllGather ships 128-wide rows.
        ha1v = ha1_sh[:].rearrange("(t p) k -> p t k", p=P)
        TA, TBB = 4, cfg.TB - 4
        with tc.tile_pool(name="p1", bufs=3) as pool, \
             tc.tile_pool(name="p1ps", bufs=4, space="PSUM") as pps:
            for b in range(cfg.NB):
                bts = slice(b * cfg.TB * P, (b + 1) * cfg.TB * P)
                xt = pool.tile([P, cfg.KC, cfg.TB, P], BF16, name="xt")
                nc.sync.dma_start(
                    xt[:], xT[:, bts].rearrange(
                        "(k p) (t n) -> p k t n", p=P, t=cfg.TB))
                pa = pps.tile([P, TA, cfg.DE1], F32, name="pa")
                pb = pps.tile([P, TBB, cfg.DE1], F32, name="pb")
                for tt in range(cfg.TB):
                    dst = pa[:, tt, :] if tt < TA else pb[:, tt - TA, :]
                    for k in range(cfg.KC):
                        nc.tensor.matmul(
                            out=dst, lhsT=xt[:, k, tt, :], rhs=w1s[:, k, :],
                            start=(k == 0), stop=(k == cfg.KC - 1))
                hab = pool.tile([P, cfg.TB, cfg.DE1], BF16, name="hab")
                nc.scalar.activation(hab[:, 0:TA, :], pa[:], Act.Copy)
                nc.scalar.activation(hab[:, TA:cfg.TB, :], pb[:], Act.Copy)
                nc.sync.dma_start(
                    ha1v[:, b * cfg.TB:(b + 1) * cfg.TB, 0:cfg.DE1], hab[:])

        nc.gpsimd.collective_compute(
            "AllGather", Alu.bypass, replica_groups=rg,
            ins=[ha1_sh[:]], outs=[ha1_full[:]])

        def edge_phase(layer: int):
            if layer == 1:
                ROW, NH, HDv, DV = cfg.ROW1, cfg.H1, cfg.HD, cfg.D1
                Tdst = ha1_sh[:, cfg.ROW1:cfg.ROW1 + NH]
                classes = ("1lo", "1hi")
                views = {"1lo": ha1_full[0:cfg.VH, 0:ROW],
                         "1hi": ha1_full[cfg.VH:cfg.V, 0:ROW]}
            else:
                ROW, NH, HDv, DV = cfg.ROW2, 1, cfg.D2, cfg.D2
                Tdst = ad2[:, 0:NH]
                classes = ("2p0", "2p1")
                views = {"2p0": ha2_full[0:cfg.VH2, 0:ROW],
                         "2p1": ha2_full[0:cfg.VH2, 64:64 + ROW]}
            RH = DV + NH
            qsrc = {classes[0]: 0, classes[1]: 1}
            qdst = {classes[0]: 2, classes[1]: 3}
            with tc.tile_pool(name=f"ep{layer}pre", bufs=1) as ppre, \
                 tc.tile_pool(name=f"ep{layer}", bufs=2) as pool, \
                 tc.tile_pool(name=f"ep{layer}n", bufs=1) as npool, \
                 tc.tile_pool(name=f"ep{layer}ps", bufs=4, space="PSUM") as pps:
                # Preload all idx/dstl tiles and run every dst-alpha gather
                # now: none of this needs the AllGather, so it hides under it.
                IW, DSTL, DGT = {}, {}, {}
                for s in classes:
                    iw = ppre.tile([P, cfg.NB, cfg.WCOLS], I16, name=f"iw{s}")
                    nc.sync.dma_start(iw[:], srcW[s][:])
                    IW[s] = iw
                    dw = ppre.tile([P, cfg.NB, cfg.WCOLS], I16, name=f"dw{s}")
                    nc.sync.dma_start(dw[:], dstW[s][:])
                    dl = ppre.tile([P, cfg.NB, cfg.KBH], F32, name=f"dl{s}")
                    nc.sync.dma_start(dl[:], dstl[s][:])
                    DSTL[s] = dl
                    dgt = ppre.tile([P, cfg.NB, cfg.KBH, NH], BF16,
                                    name=f"Dg{s}")
                    for b in range(cfg.NB):
                        dma_gather_raw(
                            nc.gpsimd, dgt[:, b], Tdst,
                            dw[:, b], cfg.NIDX, NH, P,
                            queue_num=qdst[s], single_packet=False)
                    DGT[s] = dgt
                for b in range(cfg.NB):
                    G, R = {}, {}
                    for s in classes:
                        g = pool.tile([P, cfg.KBH, ROW], BF16, name=f"G{s}")
                        dma_gather_raw(
                            nc.gpsimd, g[:], views[s],
                            IW[s][:, b], cfg.NIDX, ROW, P,
                            queue_num=qsrc[s], single_packet=False)
                        G[s] = g
                        TE = npool.tile([P, cfg.KBH, NH], F32, name=f"TE{s}")
                        nc.vector.tensor_tensor(
                            out=TE[:], in0=g[:, :, DV:DV + NH],
                            in1=DGT[s][:, b],
                            op=Alu.add)
                        # exp(leaky(x)) == max(exp(x), exp(0.2*x))
                        E1 = npool.tile([P, cfg.KBH, NH], BF16, name=f"E1{s}")
                        nc.scalar.activation(E1[:], TE[:], Act.Exp)
                        E2 = npool.tile([P, cfg.KBH, NH], BF16, name=f"E2{s}")
                        nc.scalar.activation(E2[:], TE[:], Act.Exp,
                                             scale=NEG_SLOPE)
                        r = pool.tile([P, cfg.KBH, RH], BF16, name=f"R{s}")
                        nc.vector.tensor_tensor(
                            out=r[:, :, DV:RH], in0=E1[:], in1=E2[:],
                            op=Alu.max)
                        nc.vector.tensor_tensor(
                            out=r[:, :, 0:DV].rearrange(
                                "p c (h r) -> p c h r", h=NH),
                            in0=g[:, :, 0:DV].rearrange(
                                "p c (h r) -> p c h r", h=NH),
                            in1=r[:, :, DV:RH].unsqueeze(3).broadcast_to(
                                [P, cfg.KBH, NH, HDv]),
                            op=Alu.mult)
                        R[s] = r
                    psb = npool.tile([P, cfg.TB, RH], F32, name="psb")
                    for tt in range(cfg.TB):
                        cs = slice(tt * cfg.CH, (tt + 1) * cfg.CH)
                        ps = pps.tile([P, RH], F32, name="ps")
                        for si, s in enumerate(classes):
                            oh = pool.tile([P, cfg.CH, P], BF16,
                                           name=f"oh{s}")
                            for c in range(cfg.CH):
                                nc.vector.tensor_scalar(
                                    out=oh[:, c, :], in0=iota_bf[:],
                                    scalar1=DSTL[s][
                                        :, b,
                                        tt * cfg.CH + c:tt * cfg.CH + c + 1],
                                    scalar2=None, op0=Alu.is_equal)
                            for c in range(cfg.CH):
                                nc.tensor.matmul(
                                    out=ps[:], lhsT=oh[:, c, :],
                                    rhs=R[s][:, tt * cfg.CH + c, :],
                                    start=(si == 0 and c == 0),
                                    stop=(si == 1 and c == cfg.CH - 1))
                        nc.scalar.activation(psb[:, tt, :], ps[:], Act.Copy)
                    # batched normalization over the TB tiles
                    Se = npool.tile([P, cfg.TB, NH], F32, name="Se")
                    nc.vector.tensor_scalar_add(Se[:], psb[:, :, DV:RH], 1e-30)
                    RS = npool.tile([P, cfg.TB, NH], F32, name="RS")
                    nc.vector.reciprocal(RS[:], Se[:])
                    zb = npool.tile([P, cfg.TB, DV], F32, name="zb")
                    nc.vector.tensor_tensor(
                        out=zb[:].rearrange("p t (h r) -> p t h r", h=NH),
                        in0=psb[:, :, 0:DV].rearrange(
                            "p t (h r) -> p t h r", h=NH),
                        in1=RS[:].unsqueeze(3).broadcast_to(
                            [P, cfg.TB, NH, HDv]),
                        op=Alu.mult)
                    if layer == 1:
                        zc = npool.tile([P, cfg.TB, DV], F32, name="zc")
                        nc.vector.tensor_tensor(
                            out=zc[:], in0=zb[:],
                            in1=b1_s[:].unsqueeze(1).broadcast_to(
                                [P, cfg.TB, cfg.D1]),
                            op=Alu.add)
                        mn = npool.tile([P, cfg.TB, DV], F32, name="mn")
                        nc.vector.tensor_scalar_min(mn[:], zc[:], 0.0)
                        em = npool.tile([P, cfg.TB, DV], F32, name="em")
                        nc.scalar.activation(em[:], mn[:], Act.Exp)
                        rp = npool.tile([P, cfg.TB, DV], F32, name="rp")
                        nc.vector.tensor_scalar_max(rp[:], zc[:], 0.0)
                        s1 = npool.tile([P, cfg.TB, DV], F32, name="s1")
                        nc.vector.tensor_tensor(
                            out=s1[:], in0=rp[:], in1=em[:], op=Alu.add)
                        zel = npool.tile([P, cfg.TB, DV], BF16, name="zel")
                        nc.vector.tensor_scalar_add(zel[:], s1[:], -1.0)
                        for tt in range(cfg.TB):
                            t = b * cfg.TB + tt
                            ts = slice(t * P, (t + 1) * P)
                            ztp = pps.tile([cfg.D1, P], BF16, name="ztp")
                            nc.tensor.transpose(ztp[:], zel[:, tt, :],
                                                ident[:])
                            nc.vector.tensor_copy(ZT[:, ts], ztp[:])
                    else:
                        o2 = npool.tile([P, cfg.TB, DV], F32, name="o2")
                        nc.vector.tensor_tensor(
                            out=o2[:], in0=zb[:],
                            in1=b2_s[:].unsqueeze(1).broadcast_to(
                                [P, cfg.TB, cfg.D2]),
                            op=Alu.add)
                        nc.sync.dma_start(
                            out[:].rearrange("(t p) d -> p t d", p=P)[
                                :, b * cfg.TB:(b + 1) * cfg.TB, :],
                            o2[:])

        edge_phase(1)

        # ---------------- Phase 4: [h2 | as2 | ad2] = z @ W2cat ----------------
        ha2v = ha2_sh[:].rearrange("r (s k) -> (r s) k", s=2)
        ha2vb = ha2v.rearrange("(t p) k -> p t k", p=P)
        ad2v = ad2[:].rearrange("(t p) k -> p t k", p=P)
        with tc.tile_pool(name="p4", bufs=3) as pool, \
             tc.tile_pool(name="p4ps", bufs=4, space="PSUM") as pps:
            for b in range(cfg.NB):
                h2ps = pps.tile([P, cfg.TB, cfg.DE2], F32, name="h2ps")
                for tt in range(cfg.TB):
                    t = b * cfg.TB + tt
                    nc.tensor.matmul(
                        out=h2ps[:, tt, :], lhsT=ZT[:, t * P:(t + 1) * P],
                        rhs=w2s[:], start=True, stop=True)
                ha2 = pool.tile([P, cfg.TB, cfg.ROW2], BF16, name="ha2")
                nc.scalar.activation(ha2[:], h2ps[:, :, 0:cfg.ROW2], Act.Copy)
                nc.sync.dma_start(
                    ha2vb[:, b * cfg.TB:(b + 1) * cfg.TB, 0:cfg.ROW2], ha2[:])
                ad2b = pool.tile([P, cfg.TB, 1], BF16, name="ad2b")
                nc.scalar.activation(ad2b[:], h2ps[:, :, cfg.ROW2:cfg.DE2],
                                     Act.Copy)
                nc.sync.dma_start(
                    ad2v[:, b * cfg.TB:(b + 1) * cfg.TB, 0:1], ad2b[:])

        nc.gpsimd.collective_compute(
            "AllGather", Alu.bypass, replica_groups=rg,
            ins=[ha2_sh[:]], outs=[ha2_full[:]])

        edge_phase(2)
        cpool_cm.__exit__(None, None, None)

    nc.compile()
    return nc


# ---------------- host-side preprocessing ----------------

def _wrap16(idx):
    n = idx.shape[0]
    w = idx.reshape(n // 16, 16).T.astype(np.int16)
    return np.tile(w, (8, 1))                      # [128, n/16]


def _group_edges(cfg: Cfg, core, tl, loc, row, cls):
    """Group edges by (core, tile, cls); within group order by row."""
    gid = (core * cfg.TILES + tl) * 2 + cls
    order = np.lexsort((row, gid))
    gid, row, loc = gid[order], row[order], loc[order]
    counts = np.bincount(gid, minlength=cfg.NC * cfg.TILES * 2)
    assert counts.max() <= cfg.CH * P, (counts.max(), cfg.CH * P)
    starts = np.zeros(len(counts) + 1, dtype=np.int64)
    np.cumsum(counts, out=starts[1:])
    pos = np.arange(len(gid)) - starts[gid]

    CHP = cfg.CH * P
    shape = (cfg.NC, cfg.TILES, 2, CHP)
    src_pad = np.zeros(shape, dtype=np.int32)
    dloc_pad = np.zeros(shape, dtype=np.int32)
    dstl_pad = np.full(shape, P, dtype=np.float32)
    c_ = gid // (cfg.TILES * 2)
    t_ = (gid // 2) % cfg.TILES
    h_ = gid % 2
    src_pad[c_, t_, h_, pos] = row.astype(np.int32)
    dloc_pad[c_, t_, h_, pos] = loc.astype(np.int32)
    dstl_pad[c_, t_, h_, pos] = (loc % P).astype(np.float32)

    outs = []
    for hi in range(2):
        sW = np.zeros((cfg.NC, P, cfg.NB, cfg.WCOLS), dtype=np.int16)
        dW = np.zeros((cfg.NC, P, cfg.NB, cfg.WCOLS), dtype=np.int16)
        dL = np.zeros((cfg.NC, P, cfg.NB, cfg.KBH), dtype=np.float32)
        for c in range(cfg.NC):
            for b in range(cfg.NB):
                tt0 = b * cfg.TB
                sv = src_pad[c, tt0:tt0 + cfg.TB, hi].ravel()
                dv = dloc_pad[c, tt0:tt0 + cfg.TB, hi].ravel()
                sW[c, :, b] = _wrap16(sv)
                dW[c, :, b] = _wrap16(dv)
                # dstl in chunk-major lanes: [TB, CH, P] -> [P, TB*CH]
                dL[c, :, b] = dstl_pad[c, tt0:tt0 + cfg.TB, hi].reshape(
                    cfg.TB * cfg.CH, P).T
        outs.append((np.ascontiguousarray(sW), np.ascontiguousarray(dW),
                     np.ascontiguousarray(dL)))
    return outs


def preprocess_edges(edge_index: np.ndarray, cfg: Cfg):
    N = cfg.N
    src = np.concatenate([np.asarray(edge_index[0]).astype(np.int64),
                          np.arange(N, dtype=np.int64)])
    dst = np.concatenate([np.asarray(edge_index[1]).astype(np.int64),
                          np.arange(N, dtype=np.int64)])
    core = dst // cfg.SHARD
    loc = dst % cfg.SHARD
    tl = loc // P
    # layer 1: lo/hi halves of the [V, 128] table
    src_remap = (src // cfg.SHARD) * cfg.SHARD_PAD + (src % cfg.SHARD)
    cls1 = (src_remap >= cfg.VH).astype(np.int64)
    row1 = src_remap - cls1 * cfg.VH
    l1 = _group_edges(cfg, core, tl, loc, row1, cls1)
    # layer 2: packed [V/2, 128] table, class = src parity
    row2 = (src // cfg.SHARD) * cfg.SHP2 + (src % cfg.SHARD) // 2
    cls2 = src % 2
    l2 = _group_edges(cfg, core, tl, loc, row2, cls2)
    return {"1lo": l1[0], "1hi": l1[1], "2p0": l2[0], "2p1": l2[1]}


def make_in_maps(inputs: dict, cfg: Cfg):
    x = np.asarray(inputs["x"], dtype=np.float32)
    ei = np.asarray(inputs["edge_index"]).astype(np.int64)
    W1 = np.asarray(inputs["W1"], dtype=np.float32)
    a1_src = np.asarray(inputs["a1_src"], dtype=np.float32)
    a1_dst = np.asarray(inputs["a1_dst"], dtype=np.float32)
    b1 = np.asarray(inputs["b1"], dtype=np.float32)
    W2 = np.asarray(inputs["W2"], dtype=np.float32)
    a2_src = np.asarray(inputs["a2_src"], dtype=np.float32)
    a2_dst = np.asarray(inputs["a2_dst"], dtype=np.float32)
    b2 = np.asarray(inputs["b2"], dtype=np.float32)

    ed = preprocess_edges(ei, cfg)
    # Fold the attention projections into the dense weights:
    # A1s [D1, H1] with A1s[h*HD+d, h] = a1_src[h, d] (block diagonal).
    A1s = np.zeros((cfg.D1, cfg.H1), np.float32)
    A1d = np.zeros((cfg.D1, cfg.H1), np.float32)
    for h in range(cfg.H1):
        A1s[h * cfg.HD:(h + 1) * cfg.HD, h] = a1_src[h]
        A1d[h * cfg.HD:(h + 1) * cfg.HD, h] = a1_dst[h]
    W1cat = np.concatenate([W1, W1 @ A1s, W1 @ A1d], axis=1)  # [F, DE1]
    W2cat = np.concatenate(
        [W2, W2 @ a2_src.reshape(cfg.D2, 1), W2 @ a2_dst.reshape(cfg.D2, 1)],
        axis=1)                                               # [D1, DE2]
    w1_dev = np.ascontiguousarray(
        W1cat.reshape(cfg.KC, P, cfg.DE1).transpose(1, 0, 2)
        .reshape(P, cfg.KC * cfg.DE1)).astype(BF)
    consts = {
        "w1": w1_dev, "w2": W2cat.astype(BF),
        "b1r": np.broadcast_to(b1.reshape(1, cfg.D1), (P, cfg.D1)).copy(),
        "b2r": np.broadcast_to(b2.reshape(1, cfg.D2), (P, cfg.D2)).copy(),
    }
    in_maps = []
    for c in range(cfg.NC):
        xs = x[c * cfg.SHARD:(c + 1) * cfg.SHARD]
        xTc = np.zeros((cfg.F, cfg.SHARD_PAD), dtype=BF)
        xTc[:, :cfg.SHARD] = xs.T.astype(BF)
        m = {"xT": xTc, **consts}
        for k, (sW, dW, dL) in ed.items():
            m[f"srcW_{k}"] = sW[c]
            m[f"dstW_{k}"] = dW[c]
            m[f"dstl_{k}"] = dL[c]
        in_maps.append(m)
    return in_maps


def assemble_output(results, cfg: Cfg):
    outs = [results[c]["out"][:cfg.SHARD] for c in range(cfg.NC)]
    return np.concatenate(outs, axis=0).astype(np.float32)


def pick_ch(edge_index: np.ndarray, cfg_kwargs: dict) -> int:
    tmp = Cfg(CH=1, TB=1, **{k: v for k, v in cfg_kwargs.items()
                             if k in ("N", "NC", "F", "H1", "HD", "D2")})
    N = tmp.N
    src = np.concatenate([np.asarray(edge_index[0]).astype(np.int64),
                          np.arange(N, dtype=np.int64)])
    dst = np.concatenate([np.asarray(edge_index[1]).astype(np.int64),
                          np.arange(N, dtype=np.int64)])
    base = (dst // tmp.SHARD) * tmp.TILES + (dst % tmp.SHARD) // P
    src_remap = (src // tmp.SHARD) * tmp.SHARD_PAD + (src % tmp.SHARD)
    mx = 0
    for cls in ((src_remap >= tmp.VH).astype(np.int64), src % 2):
        counts = np.bincount(base * 2 + cls,
                             minlength=tmp.NC * tmp.TILES * 2)
        mx = max(mx, int(counts.max()))
    return int(math.ceil(mx / P))


# ---------------- public entry point ----------------

_CACHE = {}


def kernel(**inputs) -> np.ndarray:
    ei = np.asarray(inputs["edge_index"]).astype(np.int64)
    ch = max(10, pick_ch(ei, dict(N=50000, NC=8, F=512)))
    cfg = Cfg(N=50000, NC=8, F=512, CH=ch, TB=7)
    key = ch
    if key not in _CACHE:
        _CACHE[key] = build_program(cfg)
    nc = _CACHE[key]
    in_maps = make_in_maps(inputs, cfg)
    from concourse import bass_utils
    res = bass_utils.run_bass_kernel_spmd(
        nc, in_maps, core_ids=list(range(cfg.NC)))
    return assemble_output(res.results, cfg)


# revision 38
# speedup vs baseline: 1.1398x; 1.1398x over previous
"""Trainium2 Bass kernel for a 2-layer GAT (nn_GAT_50586124812836).

kernel(**inputs) takes the FULL inputs from reference.setup_inputs() and
returns the full [50000, 32] float32 output.

Design (destination-node sharding across 8 NeuronCores):
- Dense phases compute [h | alpha_src | alpha_dst] in one PE matmul per
  tile (attention projections folded into the weights host-side).
- An AllGather publishes a packed bf16 node table per layer; layer 2
  packs 2 nodes per 256B row, halving that collective.
- The edge phase gathers source-node rows with SWDGE dma_gather (int16
  indices; the serial cost on hardware is ~8ns per descriptor, so
  descriptor count is what we minimize):
  * exact per-(tile,class) chunk counts (shared across cores) instead of
    a uniform worst-case pad,
  * the appended self-loop edges never touch the gather: their rows are
    local and contiguous, added straight into the per-tile PSUM result,
  * per-edge alpha_dst is computed on-chip (transposed one-hot x ad-table
    matmul) instead of a second 16B-per-edge gather.
- Segment softmax + weighted aggregation run on-chip: exp(leaky(x)) =
  max(exp(x), exp(0.2x)) on the Activation engine, one-hot scatter
  matmuls on the PE.
"""
import math
from dataclasses import dataclass

import numpy as np
import ml_dtypes

import concourse.mybir as mybir
from concourse.bass import AP, MemorySpace
from concourse import ap_utils
from concourse._compat import exact_div

import concourse.tile as tile
from concourse import bacc
from concourse.masks import make_identity
from concourse.library_config import mlp

BF16 = mybir.dt.bfloat16
F32 = mybir.dt.float32
I16 = mybir.dt.int16
P = 128
Alu = mybir.AluOpType
Act = mybir.ActivationFunctionType
NEG_SLOPE = 0.2
BF = ml_dtypes.bfloat16


def dma_gather_raw(
    gp,                       # nc.gpsimd
    out_ap: AP,
    in_ap: AP,
    idxs_ap: AP,
    num_idxs: int,
    elem_size: int,
    elem_step: int,
    queue_num: int = 0,
    single_packet: bool = True,
):
    assert idxs_ap.dtype == mybir.dt.int16
    assert in_ap.space == MemorySpace.DRAM
    assert idxs_ap.space == MemorySpace.SBUF
    assert out_ap.space == MemorySpace.SBUF
    assert in_ap.dtype == out_ap.dtype
    dtsz = mybir.dt.size(in_ap.dtype)
    stride_bytes = elem_step * dtsz
    stride_bytes_256 = exact_div(stride_bytes, 256)
    assert 0 < stride_bytes_256 < 256
    assert ap_utils.ap_is_contiguous(in_ap.ap[1:])
    assert ap_utils.ap_is_contiguous(out_ap.ap[1:])
    assert ap_utils.ap_is_contiguous(idxs_ap.ap[1:])
    assert in_ap.ap[0][0] == elem_step
    assert in_ap.ap[-1][1] == elem_size
    assert out_ap.ap[-1][1] == elem_size
    assert num_idxs % 128 == 0
    assert out_ap.ap[0][1] * out_ap.ap[1][1] == num_idxs

    _in_ap = gp.lower_ap_dma(in_ap, for_custom_bir_dma=True)
    _idxs_ap = gp.lower_ap(idxs_ap)
    _out_ap = gp.lower_ap(out_ap)
    inst = gp.add_instruction(
        mybir.InstDMAGatherAnt(
            name=gp.bass.get_next_instruction_name(),
            ins=[
                *_in_ap,
                _idxs_ap,
                gp.lower_val_access(gp.to_reg(num_idxs)),
            ],
            outs=[_out_ap],
            transpose=False,
            num_idxs=num_idxs,
            elem_size=elem_size,
            stride_bytes_256=stride_bytes_256,
            gen_mode=0,
            single_packet=single_packet,
            queue_num=queue_num,
            sbuf_tokens_per_rank=0,
            sbuf_free_dim_per_rank=0,
            sbuf_free_dim_pad_per_rank=0,
            sbuf_byte_offset=0,
        )
    )
    return inst


@dataclass
class Cfg:
    N: int = 50000
    NC: int = 8
    F: int = 512
    H1: int = 8
    HD: int = 8
    D2: int = 32
    TB: int = 7          # tiles per batch

    def __post_init__(self):
        self.D1 = self.H1 * self.HD
        assert self.N % self.NC == 0
        self.SHARD = self.N // self.NC
        self.TILES = math.ceil(self.SHARD / P)
        self.SHARD_PAD = self.TILES * P
        self.V = self.NC * self.SHARD_PAD
        self.VH = self.V // 2          # layer-1 lo/hi table split
        self.VH2 = self.V // 2         # layer-2 packed table rows
        assert self.VH < 32768
        assert self.TILES % self.TB == 0
        self.NB = self.TILES // self.TB
        assert self.F % P == 0
        self.KC = self.F // P
        self.ROW1 = self.D1 + self.H1           # 72
        self.ROW2 = self.D2 + 1                 # 33
        self.DE1 = self.D1 + 2 * self.H1        # 80: [h | as | ad]
        self.DE2 = self.D2 + 2                  # 34: [h2 | as2 | ad2]
        assert self.SHARD_PAD % 2 == 0
        self.SHP2 = self.SHARD_PAD // 2


CLASSES = ("1lo", "1hi", "2p0", "2p1")


def build_plan(edge_index: np.ndarray, cfg: Cfg):
    """Exact per-(tile, class) chunk counts, shared across cores (max).

    Self-loops appended by the reference are handled separately on-chip
    and excluded here. Natural (i,i) edges stay in the normal path.
    """
    src = np.asarray(edge_index[0]).astype(np.int64)
    dst = np.asarray(edge_index[1]).astype(np.int64)
    core = dst // cfg.SHARD
    tl = (dst % cfg.SHARD) // P
    src_remap = (src // cfg.SHARD) * cfg.SHARD_PAD + (src % cfg.SHARD)
    cls = {"1": (src_remap >= cfg.VH).astype(np.int64), "2": src % 2}
    plan = {}
    for li in ("1", "2"):
        gid = (core * cfg.TILES + tl) * 2 + cls[li]
        counts = np.bincount(gid, minlength=cfg.NC * cfg.TILES * 2)
        counts = counts.reshape(cfg.NC, cfg.TILES, 2).max(axis=0)
        for ci, s in enumerate(("lo", "hi") if li == "1" else ("p0", "p1")):
            cht = np.ceil(counts[:, ci] / P).astype(int)
            choff = np.zeros(cfg.TILES + 1, dtype=int)
            np.cumsum(cht, out=choff[1:])
            plan[li + s] = {
                "cht": [int(x) for x in cht],
                "choff": [int(x) for x in choff],
                "tot": int(choff[-1]),
            }
    return plan


def plan_key(plan):
    return tuple((k, tuple(plan[k]["cht"])) for k in CLASSES)


def build_program(cfg: Cfg, plan):
    nc = bacc.Bacc("TRN2", target_bir_lowering=False, debug=False,
                   num_devices=cfg.NC, num_swdge_queues=4)
    dt = nc.dram_tensor
    xT = dt("xT", [cfg.F, cfg.SHARD_PAD], BF16, kind="ExternalInput")
    w1 = dt("w1", [P, cfg.KC * cfg.DE1], BF16, kind="ExternalInput")
    w2 = dt("w2", [cfg.D1, cfg.DE2], BF16, kind="ExternalInput")
    b1r = dt("b1r", [P, cfg.D1], F32, kind="ExternalInput")
    b2r = dt("b2r", [P, cfg.D2], F32, kind="ExternalInput")
    srcW = {}
    dstl = {}
    for k in CLASSES:
        tot = plan[k]["tot"]
        srcW[k] = dt(f"srcW_{k}", [P, tot * 8], I16, kind="ExternalInput")
        dstl[k] = dt(f"dstl_{k}", [P, tot], F32, kind="ExternalInput")
    out = dt("out", [cfg.SHARD_PAD, cfg.D2], F32, kind="ExternalOutput")

    gspace = "Shared" if cfg.NC > 4 else "Local"
    ha1_sh = dt("ha1_sh", [cfg.SHARD_PAD, P], BF16, kind="Internal")
    ha1_full = dt("ha1_full", [cfg.V, P], BF16, kind="Internal",
                  addr_space=gspace)
    ha2_sh = dt("ha2_sh", [cfg.SHP2, P], BF16, kind="Internal")
    ha2_full = dt("ha2_full", [cfg.VH2, P], BF16, kind="Internal",
                  addr_space=gspace)
    ad2 = dt("ad2", [cfg.SHARD_PAD, P], BF16, kind="Internal")

    rg = [list(range(cfg.NC))]

    with tile.TileContext(nc) as tc:
        cpool_cm = tc.tile_pool(name="consts", bufs=1)
        cpool = cpool_cm.__enter__()
        nc.gpsimd.load_library(mlp)
        w1s = cpool.tile([P, cfg.KC, cfg.DE1], BF16)
        nc.sync.dma_start(w1s[:], w1[:].rearrange("p (k d) -> p k d", k=cfg.KC))
        w2s = cpool.tile([cfg.D1, cfg.DE2], BF16)
        nc.sync.dma_start(w2s[:], w2[:])
        b1_s = cpool.tile([P, cfg.D1], F32)
        nc.sync.dma_start(b1_s[:], b1r[:])
        b2_s = cpool.tile([P, cfg.D2], F32)
        nc.sync.dma_start(b2_s[:], b2r[:])
        iota_i = cpool.tile([P, P], mybir.dt.int32)
        nc.gpsimd.iota(iota_i[:], pattern=[[1, P]], base=0,
                       channel_multiplier=0)
        iota_bf = cpool.tile([P, P], BF16)
        nc.vector.tensor_copy(iota_bf[:], iota_i[:])
        ident = cpool.tile([P, P], BF16)
        make_identity(nc, ident[:])
        ZT = cpool.tile([cfg.D1, cfg.TILES * P], BF16)

        # ------- Phase 1: [h1 | as1 | ad1] = x @ W1cat, batched per TB -------
        # ha1_sh row layout: [h(64) | as(8) | ad(8) | pad].
        ha1v = ha1_sh[:].rearrange("(t p) k -> p t k", p=P)
        TA, TBB = 4, cfg.TB - 4
        with tc.tile_pool(name="p1", bufs=3) as pool, \
             tc.tile_pool(name="p1ps", bufs=4, space="PSUM") as pps:
            for b in range(cfg.NB):
                bts = slice(b * cfg.TB * P, (b + 1) * cfg.TB * P)
                xt = pool.tile([P, cfg.KC, cfg.TB, P], BF16, name="xt")
                nc.sync.dma_start(
                    xt[:], xT[:, bts].rearrange(
                        "(k p) (t n) -> p k t n", p=P, t=cfg.TB))
                pa = pps.tile([P, TA, cfg.DE1], F32, name="pa")
                pb = pps.tile([P, TBB, cfg.DE1], F32, name="pb")
                for tt in range(cfg.TB):
                    dst = pa[:, tt, :] if tt < TA else pb[:, tt - TA, :]
                    for k in range(cfg.KC):
                        nc.tensor.matmul(
                            out=dst, lhsT=xt[:, k, tt, :], rhs=w1s[:, k, :],
                            start=(k == 0), stop=(k == cfg.KC - 1))
                hab = pool.tile([P, cfg.TB, cfg.DE1], BF16, name="hab")
                nc.scalar.activation(hab[:, 0:TA, :], pa[:], Act.Copy)
                nc.scalar.activation(hab[:, TA:cfg.TB, :], pb[:], Act.Copy)
                nc.sync.dma_start(
                    ha1v[:, b * cfg.TB:(b + 1) * cfg.TB, 0:cfg.DE1], hab[:])

        nc.gpsimd.collective_compute(
            "AllGather", Alu.bypass, replica_groups=rg,
            ins=[ha1_sh[:]], outs=[ha1_full[:]])

        ha2v = ha2_sh[:].rearrange("r (s k) -> (r s) k", s=2)
        ha2vb = ha2v.rearrange("(t p) k -> p t k", p=P)
        ad2v = ad2[:].rearrange("(t p) k -> p t k", p=P)

        def edge_phase(layer: int):
            if layer == 1:
                ROW, NH, HDv, DV = cfg.ROW1, cfg.H1, cfg.HD, cfg.D1
                Tdst = ha1v[:, :, cfg.ROW1:cfg.ROW1 + NH]
                classes = ("1lo", "1hi")
                views = {"1lo": ha1_full[0:cfg.VH, 0:ROW],
                         "1hi": ha1_full[cfg.VH:cfg.V, 0:ROW]}
            else:
                ROW, NH, HDv, DV = cfg.ROW2, 1, cfg.D2, cfg.D2
                Tdst = ad2v[:, :, 0:NH]
                classes = ("2p0", "2p1")
                views = {"2p0": ha2_full[0:cfg.VH2, 0:ROW],
                         "2p1": ha2_full[0:cfg.VH2, 64:64 + ROW]}
            RH = DV + NH
            qsrc = {classes[0]: 0, classes[1]: 1}
            CHT = {s: plan[s]["cht"] for s in classes}
            CHOFF = {s: plan[s]["choff"] for s in classes}
            TOT = {s: plan[s]["tot"] for s in classes}
            chunk_tile = {s: [t for t in range(cfg.TILES)
                              for _ in range(CHT[s][t])] for s in classes}
            with tc.tile_pool(name=f"ep{layer}pre", bufs=1) as ppre, \
                 tc.tile_pool(name=f"ep{layer}", bufs=2) as pool, \
                 tc.tile_pool(name=f"ep{layer}n", bufs=1) as npool, \
                 tc.tile_pool(name=f"ep{layer}d", bufs=3) as dpool, \
                 tc.tile_pool(name=f"ep{layer}ps", bufs=2, space="PSUM") as pps, \
                 tc.tile_pool(name=f"ep{layer}pd", bufs=2,
                              space="PSUM") as ppsd:
                # Preload idx/dstl tiles, then compute every per-edge
                # dst-alpha on-chip; none of this needs the AllGather, so it
                # all hides under it.
                IW, DSTL, DGT = {}, {}, {}
                for s in classes:
                    iw = ppre.tile([P, TOT[s] * 8], I16, name=f"iw{s}")
                    nc.sync.dma_start(iw[:], srcW[s][:])
                    IW[s] = iw
                    dl = ppre.tile([P, TOT[s]], F32, name=f"dl{s}")
                    nc.sync.dma_start(dl[:], dstl[s][:])
                    DSTL[s] = dl
                ad_sb = ppre.tile([P, cfg.TILES, NH], BF16, name="ad_sb")
                nc.sync.dma_start(ad_sb[:], Tdst)
                G4 = 4
                for s in classes:
                    dgt = ppre.tile([P, TOT[s], NH], BF16, name=f"Dg{s}")
                    for c0 in range(0, TOT[s], G4):
                        ng = min(G4, TOT[s] - c0)
                        psD = ppsd.tile([P, G4, NH], F32, name="psD")
                        ohc = dpool.tile([P, G4, P], BF16, name="ohc")
                        for ci in range(ng):
                            nc.vector.tensor_scalar(
                                out=ohc[:, ci, :], in0=iota_bf[:],
                                scalar1=DSTL[s][:, c0 + ci:c0 + ci + 1],
                                scalar2=None, op0=Alu.is_equal)
                        ohT_ps = ppsd.tile([P, G4, P], BF16, name="ohT_ps")
                        for ci in range(ng):
                            nc.tensor.transpose(ohT_ps[:, ci, :],
                                                ohc[:, ci, :], ident[:])
                        ohT = dpool.tile([P, G4, P], BF16, name="ohT")
                        nc.vector.tensor_copy(ohT[:, 0:ng, :],
                                              ohT_ps[:, 0:ng, :])
                        for ci in range(ng):
                            nc.tensor.matmul(
                                out=psD[:, ci, :], lhsT=ohT[:, ci, :],
                                rhs=ad_sb[:, chunk_tile[s][c0 + ci], :],
                                start=True, stop=True)
                        nc.scalar.activation(
                            dgt[:, c0:c0 + ng, :], psD[:, 0:ng, :], Act.Copy)
                    DGT[s] = dgt
                for b in range(cfg.NB):
                    t0, t1 = b * cfg.TB, (b + 1) * cfg.TB
                    G, R, C0 = {}, {}, {}
                    for s in classes:
                        c0, c1 = CHOFF[s][t0], CHOFF[s][t1]
                        kb = c1 - c0
                        C0[s] = c0
                        g = pool.tile([P, kb, ROW], BF16, name=f"G{s}")
                        dma_gather_raw(
                            nc.gpsimd, g[:], views[s],
                            IW[s][:, c0 * 8:c1 * 8], kb * P, ROW, P,
                            queue_num=qsrc[s] * 2 + b % 2,
                            single_packet=False)
                        G[s] = g
                        TE = npool.tile([P, kb, NH], F32, name=f"TE{s}")
                        nc.vector.tensor_tensor(
                            out=TE[:], in0=g[:, :, DV:DV + NH],
                            in1=DGT[s][:, c0:c1, :], op=Alu.add)
                        # exp(leaky(x)) == max(exp(x), exp(0.2*x))
                        E1 = npool.tile([P, kb, NH], BF16, name=f"E1{s}")
                        nc.scalar.activation(E1[:], TE[:], Act.Exp)
                        E2 = npool.tile([P, kb, NH], BF16, name=f"E2{s}")
                        nc.scalar.activation(E2[:], TE[:], Act.Exp,
                                             scale=NEG_SLOPE)
                        r = pool.tile([P, kb, RH], BF16, name=f"R{s}")
                        nc.vector.tensor_tensor(
                            out=r[:, :, DV:RH], in0=E1[:], in1=E2[:],
                            op=Alu.max)
                        nc.vector.tensor_tensor(
                            out=r[:, :, 0:DV].rearrange(
                                "p c (h r) -> p c h r", h=NH),
                            in0=g[:, :, 0:DV].rearrange(
                                "p c (h r) -> p c h r", h=NH),
                            in1=r[:, :, DV:RH].unsqueeze(3).broadcast_to(
                                [P, kb, NH, HDv]),
                            op=Alu.mult)
                        R[s] = r
                    # self-loop contributions: local rows, no gather
                    if layer == 1:
                        selfb = pool.tile([P, cfg.TB, cfg.DE1], BF16,
                                          name="selfb")
                        nc.sync.dma_start(
                            selfb[:], ha1v[:, t0:t1, 0:cfg.DE1])
                        sh = selfb[:, :, 0:DV]
                        TEs_in0 = selfb[:, :, DV:DV + NH]
                        TEs_in1 = selfb[:, :, cfg.ROW1:cfg.ROW1 + NH]
                    else:
                        selfb = pool.tile([P, cfg.TB, cfg.ROW2], BF16,
                                          name="selfb")
                        nc.sync.dma_start(
                            selfb[:], ha2vb[:, t0:t1, 0:cfg.ROW2])
                        sad = pool.tile([P, cfg.TB, 1], BF16, name="sad")
                        nc.sync.dma_start(sad[:], ad2v[:, t0:t1, 0:1])
                        sh = selfb[:, :, 0:DV]
                        TEs_in0 = selfb[:, :, DV:DV + NH]
                        TEs_in1 = sad[:]
                    TEs = npool.tile([P, cfg.TB, NH], F32, name="TEs")
                    nc.vector.tensor_tensor(
                        out=TEs[:], in0=TEs_in0, in1=TEs_in1, op=Alu.add)
                    E1s = npool.tile([P, cfg.TB, NH], BF16, name="E1s")
                    nc.scalar.activation(E1s[:], TEs[:], Act.Exp)
                    E2s = npool.tile([P, cfg.TB, NH], BF16, name="E2s")
                    nc.scalar.activation(E2s[:], TEs[:], Act.Exp,
                                         scale=NEG_SLOPE)
                    rs = npool.tile([P, cfg.TB, RH], F32, name="rs")
                    nc.vector.tensor_tensor(
                        out=rs[:, :, DV:RH], in0=E1s[:], in1=E2s[:],
                        op=Alu.max)
                    nc.vector.tensor_tensor(
                        out=rs[:, :, 0:DV].rearrange(
                            "p t (h r) -> p t h r", h=NH),
                        in0=sh.rearrange("p t (h r) -> p t h r", h=NH),
                        in1=rs[:, :, DV:RH].unsqueeze(3).broadcast_to(
                            [P, cfg.TB, NH, HDv]),
                        op=Alu.mult)
                    psb = npool.tile([P, cfg.TB, RH], F32, name="psb")
                    for tt in range(cfg.TB):
                        t = t0 + tt
                        ps = pps.tile([P, RH], F32, name="ps")
                        mm = []
                        for s in classes:
                            for c in range(CHOFF[s][t], CHOFF[s][t + 1]):
                                mm.append((s, c))
                        ohs = {}
                        for s in classes:
                            nch = CHOFF[s][t + 1] - CHOFF[s][t]
                            if nch == 0:
                                continue
                            oh = pool.tile([P, nch, P], BF16, name=f"oh{s}")
                            for ci in range(nch):
                                nc.vector.tensor_scalar(
                                    out=oh[:, ci, :], in0=iota_bf[:],
                                    scalar1=DSTL[s][
                                        :, CHOFF[s][t] + ci:
                                        CHOFF[s][t] + ci + 1],
                                    scalar2=None, op0=Alu.is_equal)
                            ohs[s] = oh
                        for mi, (s, c) in enumerate(mm):
                            nc.tensor.matmul(
                                out=ps[:], lhsT=ohs[s][:, c - CHOFF[s][t], :],
                                rhs=R[s][:, c - C0[s], :],
                                start=(mi == 0), stop=(mi == len(mm) - 1))
                        # fold the self-loop edge in while leaving PSUM
                        nc.vector.tensor_tensor(
                            out=psb[:, tt, :], in0=ps[:], in1=rs[:, tt, :],
                            op=Alu.add)
                    # batched normalization over the TB tiles
                    Se = npool.tile([P, cfg.TB, NH], F32, name="Se")
                    nc.vector.tensor_scalar_add(Se[:], psb[:, :, DV:RH], 1e-30)
                    RS = npool.tile([P, cfg.TB, NH], F32, name="RS")
                    nc.vector.reciprocal(RS[:], Se[:])
                    zb = npool.tile([P, cfg.TB, DV], F32, name="zb")
                    nc.vector.tensor_tensor(
                        out=zb[:].rearrange("p t (h r) -> p t h r", h=NH),
                        in0=psb[:, :, 0:DV].rearrange(
                            "p t (h r) -> p t h r", h=NH),
                        in1=RS[:].unsqueeze(3).broadcast_to(
                            [P, cfg.TB, NH, HDv]),
                        op=Alu.mult)
                    if layer == 1:
                        zc = npool.tile([P, cfg.TB, DV], F32, name="zc")
                        nc.vector.tensor_tensor(
                            out=zc[:], in0=zb[:],
                            in1=b1_s[:].unsqueeze(1).broadcast_to(
                                [P, cfg.TB, cfg.D1]),
                            op=Alu.add)
                        mn = npool.tile([P, cfg.TB, DV], F32, name="mn")
                        nc.vector.tensor_scalar_min(mn[:], zc[:], 0.0)
                        em = npool.tile([P, cfg.TB, DV], F32, name="em")
                        nc.scalar.activation(em[:], mn[:], Act.Exp)
                        rp = npool.tile([P, cfg.TB, DV], F32, name="rp")
                        nc.vector.tensor_scalar_max(rp[:], zc[:], 0.0)
                        s1 = npool.tile([P, cfg.TB, DV], F32, name="s1")
                        nc.vector.tensor_tensor(
                            out=s1[:], in0=rp[:], in1=em[:], op=Alu.add)
                        zel = npool.tile([P, cfg.TB, DV], BF16, name="zel")
                        nc.vector.tensor_scalar_add(zel[:], s1[:], -1.0)
                        for tt in range(cfg.TB):
                            t = t0 + tt
                            ztp = pps.tile([cfg.D1, P], BF16, name="ztp")
                            nc.tensor.transpose(ztp[:], zel[:, tt, :],
                                                ident[:])
                            nc.vector.tensor_copy(
                                ZT[:, t * P:(t + 1) * P], ztp[:])
                    else:
                        o2 = npool.tile([P, cfg.TB, DV], F32, name="o2")
                        nc.vector.tensor_tensor(
                            out=o2[:], in0=zb[:],
                            in1=b2_s[:].unsqueeze(1).broadcast_to(
                                [P, cfg.TB, cfg.D2]),
                            op=Alu.add)
                        nc.sync.dma_start(
                            out[:].rearrange("(t p) d -> p t d", p=P)[
                                :, t0:t1, :],
                            o2[:])

        edge_phase(1)

        # ------- Phase 4: [h2 | as2 | ad2] = z @ W2cat -------
        with tc.tile_pool(name="p4", bufs=3) as pool, \
             tc.tile_pool(name="p4ps", bufs=4, space="PSUM") as pps:
            for b in range(cfg.NB):
                h2ps = pps.tile([P, cfg.TB, cfg.DE2], F32, name="h2ps")
                for tt in range(cfg.TB):
                    t = b * cfg.TB + tt
                    nc.tensor.matmul(
                        out=h2ps[:, tt, :], lhsT=ZT[:, t * P:(t + 1) * P],
                        rhs=w2s[:], start=True, stop=True)
                ha2 = pool.tile([P, cfg.TB, cfg.ROW2], BF16, name="ha2")
                nc.scalar.activation(ha2[:], h2ps[:, :, 0:cfg.ROW2], Act.Copy)
                nc.sync.dma_start(
                    ha2vb[:, b * cfg.TB:(b + 1) * cfg.TB, 0:cfg.ROW2], ha2[:])
                ad2b = pool.tile([P, cfg.TB, 1], BF16, name="ad2b")
                nc.scalar.activation(ad2b[:], h2ps[:, :, cfg.ROW2:cfg.DE2],
                                     Act.Copy)
                nc.sync.dma_start(
                    ad2v[:, b * cfg.TB:(b + 1) * cfg.TB, 0:1], ad2b[:])

        nc.gpsimd.collective_compute(
            "AllGather", Alu.bypass, replica_groups=rg,
            ins=[ha2_sh[:]], outs=[ha2_full[:]])

        edge_phase(2)
        cpool_cm.__exit__(None, None, None)

    nc.compile()
    return nc


# ---------------- host-side preprocessing ----------------

def _wrap16(idx):
    n = idx.shape[0]
    w = idx.reshape(n // 16, 16).T.astype(np.int16)
    return np.tile(w, (8, 1))                      # [128, n/16]


def preprocess_edges(edge_index: np.ndarray, cfg: Cfg, plan):
    src = np.asarray(edge_index[0]).astype(np.int64)
    dst = np.asarray(edge_index[1]).astype(np.int64)
    core = dst // cfg.SHARD
    loc = dst % cfg.SHARD
    tl = loc // P
    src_remap = (src // cfg.SHARD) * cfg.SHARD_PAD + (src % cfg.SHARD)
    row = {"1": src_remap, "2": (src // cfg.SHARD) * cfg.SHP2
           + (src % cfg.SHARD) // 2}
    cls = {"1": (src_remap >= cfg.VH).astype(np.int64), "2": src % 2}
    sub = {"1": cfg.VH, "2": 0}
    outs = {}
    for li in ("1", "2"):
        gid = (core * cfg.TILES + tl) * 2 + cls[li]
        order = np.lexsort((row[li], gid))
        g_s, r_s, l_s = gid[order], row[li][order], loc[order]
        counts = np.bincount(gid, minlength=cfg.NC * cfg.TILES * 2)
        starts = np.zeros(len(counts) + 1, dtype=np.int64)
        np.cumsum(counts, out=starts[1:])
        pos = np.arange(len(g_s)) - starts[g_s]
        c_ = g_s // (cfg.TILES * 2)
        t_ = (g_s // 2) % cfg.TILES
        h_ = g_s % 2
        for ci, s in enumerate(("lo", "hi") if li == "1" else ("p0", "p1")):
            k = li + s
            pl = plan[k]
            tot, choff = pl["tot"], np.asarray(pl["choff"])
            assert counts.reshape(cfg.NC, cfg.TILES, 2)[:, :, ci].max(
                axis=0).max() <= max(pl["cht"] + [0]) * P or True
            idx_pad = np.zeros((cfg.NC, tot * P), dtype=np.int32)
            dl_pad = np.full((cfg.NC, tot * P), P, dtype=np.float32)
            m = h_ == ci
            slot = choff[t_[m]] * P + pos[m]
            assert (pos[m] < np.asarray(pl["cht"])[t_[m]] * P).all()
            idx_pad[c_[m], slot] = (r_s[m] - ci * sub[li]).astype(np.int32)
            dl_pad[c_[m], slot] = (l_s[m] % P).astype(np.float32)
            sW = np.zeros((cfg.NC, P, tot * 8), dtype=np.int16)
            dL = np.zeros((cfg.NC, P, tot), dtype=np.float32)
            for c in range(cfg.NC):
                sW[c] = _wrap16(idx_pad[c])
                dL[c] = dl_pad[c].reshape(tot, P).T
            outs[k] = (sW, dL)
    return outs


def make_in_maps(inputs: dict, cfg: Cfg, plan):
    x = np.asarray(inputs["x"], dtype=np.float32)
    ei = np.asarray(inputs["edge_index"]).astype(np.int64)
    W1 = np.asarray(inputs["W1"], dtype=np.float32)
    a1_src = np.asarray(inputs["a1_src"], dtype=np.float32)
    a1_dst = np.asarray(inputs["a1_dst"], dtype=np.float32)
    b1 = np.asarray(inputs["b1"], dtype=np.float32)
    W2 = np.asarray(inputs["W2"], dtype=np.float32)
    a2_src = np.asarray(inputs["a2_src"], dtype=np.float32)
    a2_dst = np.asarray(inputs["a2_dst"], dtype=np.float32)
    b2 = np.asarray(inputs["b2"], dtype=np.float32)

    ed = preprocess_edges(ei, cfg, plan)
    # Fold the attention projections into the dense weights.
    A1s = np.zeros((cfg.D1, cfg.H1), np.float32)
    A1d = np.zeros((cfg.D1, cfg.H1), np.float32)
    for h in range(cfg.H1):
        A1s[h * cfg.HD:(h + 1) * cfg.HD, h] = a1_src[h]
        A1d[h * cfg.HD:(h + 1) * cfg.HD, h] = a1_dst[h]
    W1cat = np.concatenate([W1, W1 @ A1s, W1 @ A1d], axis=1)  # [F, DE1]
    W2cat = np.concatenate(
        [W2, W2 @ a2_src.reshape(cfg.D2, 1), W2 @ a2_dst.reshape(cfg.D2, 1)],
        axis=1)                                               # [D1, DE2]
    w1_dev = np.ascontiguousarray(
        W1cat.reshape(cfg.KC, P, cfg.DE1).transpose(1, 0, 2)
        .reshape(P, cfg.KC * cfg.DE1)).astype(BF)
    consts = {
        "w1": w1_dev, "w2": W2cat.astype(BF),
        "b1r": np.broadcast_to(b1.reshape(1, cfg.D1), (P, cfg.D1)).copy(),
        "b2r": np.broadcast_to(b2.reshape(1, cfg.D2), (P, cfg.D2)).copy(),
    }
    in_maps = []
    for c in range(cfg.NC):
        xs = x[c * cfg.SHARD:(c + 1) * cfg.SHARD]
        xTc = np.zeros((cfg.F, cfg.SHARD_PAD), dtype=BF)
        xTc[:, :cfg.SHARD] = xs.T.astype(BF)
        m = {"xT": xTc, **consts}
        for k, (sW, dL) in ed.items():
            m[f"srcW_{k}"] = sW[c]
            m[f"dstl_{k}"] = dL[c]
        in_maps.append(m)
    return in_maps


def assemble_output(results, cfg: Cfg):
    outs = [results[c]["out"][:cfg.SHARD] for c in range(cfg.NC)]
    return np.concatenate(outs, axis=0).astype(np.float32)


# ---------------- public entry point ----------------

_CACHE = {}


def kernel(**inputs) -> np.ndarray:
    ei = np.asarray(inputs["edge_index"]).astype(np.int64)
    cfg = Cfg(N=50000, NC=8, F=512)
    plan = build_plan(ei, cfg)
    key = plan_key(plan)
    if key not in _CACHE:
        _CACHE[key] = build_program(cfg, plan)
    nc = _CACHE[key]
    in_maps = make_in_maps(inputs, cfg, plan)
    from concourse import bass_utils
    res = bass_utils.run_bass_kernel_spmd(
        nc, in_maps, core_ids=list(range(cfg.NC)))
    return assemble_output(res.results, cfg)
